# revision 7
# baseline (speedup 1.0000x reference)
"""DNGPU cell for Trainium2 — 8 cores data-parallel over batch, and within
each core the 4 local batches split into TWO independent pipelines of 2
batches each. The two pipelines are staggered on the PE (P0.rg, P1.rg,
P0.cand, P1.cand per step) so each pipeline's sigmoid/rmem/tanh/combine
chains run while the PE works on the other pipeline — the PE never waits
on a serial chain.

Per-pipeline layout: [C partitions, cols], col = 2 + 2*l + j (l-major,
j = local batch 0/1), 2 zero-pad cols left, 4 right (WPADP=262). Conv
taps are +-2-column shifts. chB (ch 128:192) is stored duplicated:
memB rows 0:64 = chB, rows 64:128 = chB shifted left 2 (dup col c =
main col c+2), so all contractions are full 128-row moving operands:
  M0/M1/M2 = memA[:, 2k : 2k+256]   (taps, chA)
  M3       = memB[:, 0:256]         (rows 0:64 tap0, 64:128 tap1)
  M4       = memB[:, 2:258]         (rows 64:128 tap2; rows 0:64 zero w)
rmemB carries the same dup layout (dup rows from an independent DVE mul
sBrD*memB_dup), so cand chB is the same 2-matmul form: 25 uniform
128x128xN=256 fp32r matmuls per pipeline per step (fp32r full rate at
N>=256). Tensor-tensor ops require all operands on the same partition
range, so the whole B-side combine runs on partitions 64:128 (uB, qB,
nB-dup-sub all at base 64, fed by the merged hi sigmoid sBG whose rows
64:128 are gate-hi, plus a base-64 copy of reset-hi sBrD); the memB
MAIN rows are then refreshed from the dup rows by an SBUF->SBUF DMA
(idle DMA engines, unshift by 2 cols), keeping every per-step copy off
the DVE queue. Elementwise: rmemA/u muls on gpsimd, rmemB muls and
combines on DVE.
"""

import numpy as np
from contextlib import ExitStack

import concourse.bacc as bacc
import concourse.tile as tile
from concourse import mybir
from concourse.bass_utils import run_bass_kernel_spmd

B, L, C = 32, 128, 192
NCORES = 8
BLOC = B // NCORES          # 4 batches per core
NP = 2                      # pipelines per core
TOKP = 2 * L                # 256 tokens per pipeline
WPADP = TOKP + 6            # 2 zero cols left, 4 right
STEPS = 128

F32 = mybir.dt.float32
F32R = mybir.dt.float32r
AF = mybir.ActivationFunctionType
SUB = mybir.AluOpType.subtract
MULT = mybir.AluOpType.mult


def build(steps=STEPS):
    nc = bacc.Bacc("TRN2", target_bir_lowering=False, debug=False,
                   num_devices=NCORES)
    x_d = nc.dram_tensor("x", [BLOC, L, C], F32, kind="ExternalInput").ap()
    w_d = {}
    b_d = {}
    for cv, wn, bn in (("r", "w_reset", "b_reset"),
                       ("g", "w_gate", "b_gate"),
                       ("n", "w_cand", "b_cand")):
        w_d[cv] = nc.dram_tensor(wn, [3, C, C], F32, kind="ExternalInput").ap()
        b_d[cv] = nc.dram_tensor(bn, [C], F32, kind="ExternalInput").ap()
    id_d = nc.dram_tensor("ident", [128, 128], F32, kind="ExternalInput").ap()
    out_d = nc.dram_tensor("out", [BLOC, L, C], F32, kind="ExternalOutput").ap()

    with tile.TileContext(nc) as tc, ExitStack() as ctx:
        const = ctx.enter_context(tc.tile_pool(name="const", bufs=1))
        state = ctx.enter_context(tc.tile_pool(name="state", bufs=1))
        act = ctx.enter_context(tc.tile_pool(name="act", bufs=2))
        tmp = ctx.enter_context(tc.tile_pool(name="tmp", bufs=2))
        psum = ctx.enter_context(tc.tile_pool(name="psum", bufs=1, space="PSUM"))

        # --- stationary weight tiles (shared by both pipelines) ----------
        zsrc = const.tile([64, 128], F32, tag="zsrc", name="zsrc")
        nc.gpsimd.memset(zsrc[:], 0.0)

        def load_chunk_rows(t, cv, outslice, col0, ncol):
            o0, o1 = outslice
            for c in range(3):
                nc.gpsimd.dma_start(t[c][:, col0:col0 + ncol],
                                    w_d[cv][c, 0:128, o0:o1])
            nc.gpsimd.dma_start(t[3][0:64, col0:col0 + ncol],
                                w_d[cv][0, 128:192, o0:o1])
            nc.gpsimd.dma_start(t[3][64:128, col0:col0 + ncol],
                                w_d[cv][1, 128:192, o0:o1])
            nc.gpsimd.dma_start(t[4][64:128, col0:col0 + ncol],
                                w_d[cv][2, 128:192, o0:o1])

        wt = {}
        for name in ("T0", "T1", "T2", "C0", "C1"):
            wt[name] = [const.tile([128, 128], F32R, tag=f"w{name}{c}",
                                   name=f"w{name}{c}") for c in range(5)]
            nc.vector.tensor_copy(wt[name][4][0:64, :], zsrc[:])
            if name == "C1":
                for c in range(5):
                    nc.vector.tensor_copy(wt[name][c][0:64, 64:128],
                                          zsrc[0:64, 0:64])
                    nc.vector.tensor_copy(wt[name][c][64:128, 64:128],
                                          zsrc[0:64, 0:64])
        load_chunk_rows(wt["T0"], "r", (0, 128), 0, 128)
        load_chunk_rows(wt["T1"], "g", (0, 128), 0, 128)
        load_chunk_rows(wt["T2"], "r", (128, 192), 0, 64)
        load_chunk_rows(wt["T2"], "g", (128, 192), 64, 64)
        load_chunk_rows(wt["C0"], "n", (0, 128), 0, 128)
        load_chunk_rows(wt["C1"], "n", (128, 192), 0, 64)

        # --- bias tiles --------------------------------------------------
        bA = const.tile([128, 1], F32, tag="bA")
        nc.sync.dma_start(bA[:, 0], b_d["r"][0:128])
        bG = const.tile([128, 1], F32, tag="bG")
        nc.sync.dma_start(bG[:, 0], b_d["g"][0:128])
        bB = const.tile([128, 1], F32, tag="bB")
        nc.sync.dma_start(bB[0:64, 0], b_d["r"][128:192])
        nc.sync.dma_start(bB[64:128, 0], b_d["g"][128:192])
        bCA = const.tile([128, 1], F32, tag="bCA")
        nc.sync.dma_start(bCA[:, 0], b_d["n"][0:128])
        bCB = const.tile([128, 1], F32, tag="bCB")
        nc.sync.dma_start(bCB[0:64, 0], b_d["n"][128:192])

        ident = const.tile([128, 128], F32, tag="ident")
        nc.sync.dma_start(ident[:], id_d)
        identr = const.tile([128, 128], F32R, tag="identr")
        nc.gpsimd.dma_start(identr[:], id_d)

        # --- per-pipeline state tiles ------------------------------------
        memA, memB = {}, {}
        rmemA, rmemB = {}, {}
        zf32 = state.tile([128, WPADP], F32, tag="zf32", name="zf32")
        nc.gpsimd.memset(zf32[:], 0.0)
        for p in range(NP):
            for i in range(2):
                memA[p, i] = state.tile([128, WPADP], F32R, tag=f"memA{p}{i}",
                                        name=f"memA{p}{i}")
                memB[p, i] = state.tile([128, WPADP], F32R, tag=f"memB{p}{i}",
                                        name=f"memB{p}{i}")
            rmemA[p] = state.tile([128, WPADP], F32R, tag=f"rmemA{p}",
                                  name=f"rmemA{p}")
            rmemB[p] = state.tile([128, WPADP], F32R, tag=f"rmemB{p}",
                                  name=f"rmemB{p}")
            for t in (memA[p, 0], memA[p, 1], memB[p, 0], memB[p, 1],
                      rmemA[p], rmemB[p]):
                nc.vector.tensor_copy(t[:], zf32[:])

        # --- input transform: x[b,l,c] -> mem[p][c, 2 + 2l + j] ----------
        for b in range(BLOC):
            p, j = divmod(b, 2)
            xb = tmp.tile([L, C], F32, tag="xload")
            nc.sync.dma_start(xb[:], x_d[b])
            ps = psum.tile([128, L], F32, tag="tpF32")
            nc.tensor.transpose(ps[:], xb[:, 0:128], ident[:])
            nc.vector.tensor_copy(memA[p, 0][:, 2 + j: 2 + j + 2 * L: 2], ps[:])
            ps2 = psum.tile([128, L], F32, tag="tpF32")
            nc.tensor.transpose(ps2[0:64, :], xb[:, 128:192], ident[:])
            nc.vector.tensor_copy(memB[p, 0][0:64, 2 + j: 2 + j + 2 * L: 2],
                                  ps2[0:64, :])
        for p in range(NP):
            nc.vector.tensor_copy(memB[p, 0][64:128, 0:TOKP],
                                  memB[p, 0][0:64, 2:2 + TOKP])
        # uB cols 0:2 are the zero shift-in; zero them once in both rotating
        # buffers (the per-step mul only writes cols 2:TOKP)
        for p in range(NP):
            for _ in range(2):
                uBz = tmp.tile([128, TOKP], F32R, tag=f"uB{p}", name=f"uBz{p}")
                nc.vector.tensor_copy(uBz[64:128, 0:2], zf32[64:128, 0:2])

        # --- recurrence --------------------------------------------------
        MWIN = ((0, 0), (1, 2), (2, 4))

        cur = 0
        pt = {}   # psum tiles per pipeline, this step
        sig = {}  # activation tiles per pipeline

        def emit_rg(p):
            mA, mB = memA[p, cur], memB[p, cur]
            pT0 = psum.tile([128, TOKP], F32, tag=f"pT0_{p}", name=f"pT0_{p}")
            pT1 = psum.tile([128, TOKP], F32, tag=f"pT1_{p}", name=f"pT1_{p}")
            pT2 = psum.tile([128, TOKP], F32, tag=f"pT2_{p}", name=f"pT2_{p}")
            pt[p] = (pT0, pT1, pT2)

            def full(pp, wts):
                # pp complete after 5 matmuls: 3 chA taps + 2 chB (dup form)
                for c, off in MWIN:
                    nc.tensor.matmul(pp[:], wts[c][:], mA[:, off:off + TOKP],
                                     start=(c == 0), stop=False)
                nc.tensor.matmul(pp[:], wts[3][:], mB[:, 0:TOKP],
                                 start=False, stop=False)
                nc.tensor.matmul(pp[:], wts[4][:], mB[:, 2:2 + TOKP],
                                 start=False, stop=True)

            full(pT0, wt["T0"])   # reset-lo ready first -> sA early
            full(pT2, wt["T2"])   # hi (reset|gate) second -> sBG
            full(pT1, wt["T1"])   # gate-lo last -> sG

        def emit_sig(p):
            pT0, pT1, pT2 = pt[p]
            sA = act.tile([128, TOKP], F32R, tag=f"sA{p}", name=f"sA{p}")
            nc.scalar.activation(sA[:], pT0[:], AF.Sigmoid, bias=bA[:, 0:1])
            # merged hi sigmoid: rows 0:64 = reset-hi, rows 64:128 = gate-hi
            sBG = act.tile([128, TOKP], F32R, tag=f"sBG{p}", name=f"sBG{p}")
            nc.scalar.activation(sBG[:], pT2[:], AF.Sigmoid, bias=bB[:, 0:1])
            # reset-hi again, written at base partition 64 so the rmemB dup
            # mul has all operands on partitions 64:128
            sBrD = act.tile([128, TOKP], F32R, tag=f"sBrD{p}", name=f"sBrD{p}")
            nc.scalar.activation(sBrD[64:128, :], pT2[0:64, :], AF.Sigmoid,
                                 bias=bB[0:64, 0:1])
            sG = act.tile([128, TOKP], F32R, tag=f"sG{p}", name=f"sG{p}")
            nc.scalar.activation(sG[:], pT1[:], AF.Sigmoid, bias=bG[:, 0:1])
            sig[p] = (sA, sBG, sBrD, sG)

        def emit_rmem(p):
            sA, sBG, sBrD, _ = sig[p]
            mA, mB = memA[p, cur], memB[p, cur]
            nc.gpsimd.tensor_mul(rmemA[p][:, 2:2 + TOKP], sA[:],
                                 mA[:, 2:2 + TOKP])
            # main rows: rmemB[0:64, 2+c] = sBr[c] * memB[0:64, 2+c]
            nc.vector.tensor_mul(rmemB[p][0:64, 2:2 + TOKP], sBG[0:64, :],
                                 mB[0:64, 2:2 + TOKP])
            # dup rows: rmemB[64:128, c] = sBr[c] * memB[64:128, c]
            # (memB dup col c = main col c+2, so this IS main rmem shifted)
            nc.vector.tensor_mul(rmemB[p][64:128, 0:TOKP], sBrD[64:128, :],
                                 mB[64:128, 0:TOKP])

        def emit_u(p):
            _, sBG, _, sG = sig[p]
            mA, mB = memA[p, cur], memB[p, cur]
            uA = tmp.tile([128, TOKP], F32R, tag=f"uA{p}", name=f"uA{p}")
            nc.gpsimd.tensor_mul(uA[:], sG[:], mA[:, 0:TOKP])
            # uB on partitions 64:128: uB[c] = sGb[c] * mem_main[c]; main[c]
            # = dup[c-2], cols 0:2 are the zero shift-in (memset at init)
            uB = tmp.tile([128, TOKP], F32R, tag=f"uB{p}", name=f"uB{p}")
            nc.gpsimd.tensor_mul(uB[64:128, 2:TOKP], sBG[64:128, 2:TOKP],
                                 mB[64:128, 0:TOKP - 2])
            return uA, uB

        def emit_cand(p, hi_first=False):
            pC0 = psum.tile([128, TOKP], F32, tag=f"pT0_{p}", name=f"pC0_{p}")
            pC1 = psum.tile([128, TOKP], F32, tag=f"pT1_{p}", name=f"pC1_{p}")

            def full(pp, wts):
                for c, off in MWIN:
                    nc.tensor.matmul(pp[:], wts[c][:],
                                     rmemA[p][:, off:off + TOKP],
                                     start=(c == 0), stop=False)
                nc.tensor.matmul(pp[:], wts[3][:], rmemB[p][:, 0:TOKP],
                                 start=False, stop=False)
                nc.tensor.matmul(pp[:], wts[4][:], rmemB[p][:, 2:2 + TOKP],
                                 start=False, stop=True)

            if hi_first:
                full(pC1, wt["C1"])
                full(pC0, wt["C0"])
            else:
                full(pC0, wt["C0"])
                full(pC1, wt["C1"])
            return pC0, pC1

        def emit_tanh_a(p, pC0):
            cA = act.tile([128, TOKP], F32R, tag=f"cA{p}", name=f"cA{p}")
            nc.scalar.activation(cA[:], pC0[:], AF.Tanh, bias=bCA[:, 0:1])
            return cA

        def emit_tanh_b(p, pC1):
            # written at base 64 so the B-side combine stays on 64:128
            cB = act.tile([128, TOKP], F32R, tag=f"cB{p}", name=f"cB{p}")
            nc.scalar.activation(cB[64:128, :], pC1[0:64, :], AF.Tanh,
                                 bias=bCB[0:64, 0:1])
            return cB

        def emit_combine_a(p, uA, cA):
            sG = sig[p][3]
            nA = memA[p, 1 - cur]
            qA = tmp.tile([128, TOKP], F32R, tag=f"qA{p}", name=f"qA{p}")
            nc.vector.scalar_tensor_tensor(qA[:], sG[:], 1.0, cA[:],
                                           op0=SUB, op1=MULT)
            nc.vector.tensor_sub(nA[:, 2:2 + TOKP], uA[:], qA[:])

        def emit_combine_b(p, uB, cB):
            sBG = sig[p][1]
            nB = memB[p, 1 - cur]
            qB = tmp.tile([128, TOKP], F32R, tag=f"qB{p}", name=f"qB{p}")
            nc.vector.scalar_tensor_tensor(qB[64:128, :], sBG[64:128, :], 1.0,
                                           cB[64:128, :], op0=SUB, op1=MULT)
            # write the DUP rows directly (same column indexing as main):
            nc.vector.tensor_sub(nB[64:128, 0:TOKP], uB[64:128, :],
                                 qB[64:128, :])
            # main rows from dup rows (unshift by 2 cols) on the idle DMA
            # engines, off the DVE queue: nB[0:64, c+2] = nB[64:128, c].
            nc.sync.dma_start(nB[0:64, 2:2 + TOKP], nB[64:128, 0:TOKP])

        for t in range(steps):
            emit_rg(0)
            emit_rg(1)
            emit_sig(0)
            emit_rmem(0)
            emit_sig(1)
            emit_rmem(1)
            u0 = emit_u(0)
            pC0_0, pC1_0 = emit_cand(0)
            cA0 = emit_tanh_a(0, pC0_0)
            cB0 = emit_tanh_b(0, pC1_0)
            emit_combine_a(0, u0[0], cA0)
            emit_combine_b(0, u0[1], cB0)
            u1 = emit_u(1)
            pC0_1, pC1_1 = emit_cand(1, hi_first=True)
            cB1 = emit_tanh_b(1, pC1_1)
            cA1 = emit_tanh_a(1, pC0_1)
            emit_combine_b(1, u1[1], cB1)
            emit_combine_a(1, u1[0], cA1)
            cur = 1 - cur

        # --- output transform -------------------------------------------
        for b in range(BLOC):
            p, j = divmod(b, 2)
            osb = tmp.tile([L, C], F32, tag="oload")
            ps = psum.tile([L, 128], F32R, tag="tpR")
            nc.tensor.transpose(ps[:], memA[p, cur][:, 2 + j: 2 + j + 2 * L: 2],
                                identr[:])
            nc.vector.tensor_copy(osb[:, 0:128], ps[:])
            ps2 = psum.tile([L, 128], F32R, tag="tpR")
            nc.tensor.transpose(ps2[:, 0:64],
                                memB[p, cur][0:64, 2 + j: 2 + j + 2 * L: 2],
                                identr[0:64, 0:64])
            nc.vector.tensor_copy(osb[:, 128:192], ps2[:, 0:64])
            nc.sync.dma_start(out_d[b], osb[:])

    nc.compile()
    return nc


_built = {}


def _get(steps=STEPS):
    if steps not in _built:
        _built[steps] = build(steps)
    return _built[steps]


def kernel(x, w_reset, b_reset, w_gate, b_gate, w_cand, b_cand, steps=STEPS,
           trace=False):
    nc = _get(steps)
    ident = np.eye(128, dtype=np.float32)
    base = {"w_reset": np.asarray(w_reset, np.float32),
            "b_reset": np.asarray(b_reset, np.float32),
            "w_gate": np.asarray(w_gate, np.float32),
            "b_gate": np.asarray(b_gate, np.float32),
            "w_cand": np.asarray(w_cand, np.float32),
            "b_cand": np.asarray(b_cand, np.float32),
            "ident": ident}
    x = np.asarray(x, np.float32)
    in_maps = [dict(base, x=np.ascontiguousarray(x[i * BLOC:(i + 1) * BLOC]))
               for i in range(NCORES)]
    res = run_bass_kernel_spmd(nc, in_maps, core_ids=list(range(NCORES)),
                               trace=trace)
    out = np.concatenate([res.results[i]["out"] for i in range(NCORES)], axis=0)
    if trace:
        return out, res
    return out


# revision 13
# speedup vs baseline: 1.2638x; 1.2638x over previous
"""DNGPU cell for Trainium2 — 8 cores data-parallel over batch, and within
each core the 4 local batches split into TWO independent pipelines of 2
batches each. The two pipelines are staggered on the PE (P0.rg, P1.rg,
P0.cand, P1.cand per step) so each pipeline's sigmoid/rmem/tanh/combine
chains run while the PE works on the other pipeline — the PE never waits
on a serial chain.

Per-pipeline layout: [C partitions, cols], col = 2 + 2*l + j (l-major,
j = local batch 0/1), 2 zero-pad cols left, 4 right (WPADP=262). Conv
taps are +-2-column shifts. chB (ch 128:192) is stored duplicated:
memB rows 0:64 = chB, rows 64:128 = chB shifted left 2 (dup col c =
main col c+2), so all contractions are full 128-row moving operands:
  M0/M1/M2 = memA[:, 2k : 2k+256]   (taps, chA)
  M3       = memB[:, 0:256]         (rows 0:64 tap0, 64:128 tap1)
  M4       = memB[:, 2:258]         (rows 64:128 tap2; rows 0:64 zero w)
cand chB uses three 64-contraction matmuls on the rmemB main rows (no
rmemB dup maintenance): 27 matmuls per pipeline per step, all N=256
fp32r (full rate at N>=256). Tensor-tensor ops require all operands on
the same partition range, so the whole B-side combine runs on
partitions 64:128 (uB, qB, nB-dup-sub all at base 64, fed by the merged
hi sigmoid sBG whose rows 64:128 are gate-hi); the combine writes the
memB DUP rows directly and the MAIN rows are refreshed from them by an
SBUF->SBUF DMA (DMA ports are disjoint from engine ports), keeping
every per-step copy off the DVE/GpSimd queues. Elementwise: rmemA/u
muls on gpsimd, rmemB mul and combines on DVE.
"""

import numpy as np
from contextlib import ExitStack

import concourse.bacc as bacc
import concourse.tile as tile
from concourse import mybir
from concourse.bass_utils import run_bass_kernel_spmd

B, L, C = 32, 128, 192
NCORES = 8
BLOC = B // NCORES          # 4 batches per core
NP = 2                      # pipelines per core
TOKP = 2 * L                # 256 tokens per pipeline
WPADP = TOKP + 6            # 2 zero cols left, 4 right
STEPS = 128

F32 = mybir.dt.float32
F32R = mybir.dt.float32r
AF = mybir.ActivationFunctionType
SUB = mybir.AluOpType.subtract
MULT = mybir.AluOpType.mult


def build(steps=STEPS):
    nc = bacc.Bacc("TRN2", target_bir_lowering=False, debug=False,
                   num_devices=NCORES)
    x_d = nc.dram_tensor("x", [BLOC, L, C], F32, kind="ExternalInput").ap()
    w_d = {}
    b_d = {}
    for cv, wn, bn in (("r", "w_reset", "b_reset"),
                       ("g", "w_gate", "b_gate"),
                       ("n", "w_cand", "b_cand")):
        w_d[cv] = nc.dram_tensor(wn, [3, C, C], F32, kind="ExternalInput").ap()
        b_d[cv] = nc.dram_tensor(bn, [C], F32, kind="ExternalInput").ap()
    id_d = nc.dram_tensor("ident", [128, 128], F32, kind="ExternalInput").ap()
    out_d = nc.dram_tensor("out", [BLOC, L, C], F32, kind="ExternalOutput").ap()

    with tile.TileContext(nc) as tc, ExitStack() as ctx:
        const = ctx.enter_context(tc.tile_pool(name="const", bufs=1))
        state = ctx.enter_context(tc.tile_pool(name="state", bufs=1))
        act = ctx.enter_context(tc.tile_pool(name="act", bufs=2))
        tmp = ctx.enter_context(tc.tile_pool(name="tmp", bufs=2))
        psum = ctx.enter_context(tc.tile_pool(name="psum", bufs=1, space="PSUM"))

        # --- stationary weight tiles (shared by both pipelines) ----------
        zsrc = const.tile([64, 128], F32, tag="zsrc", name="zsrc")
        nc.gpsimd.memset(zsrc[:], 0.0)

        def load_chunk_rows(t, cv, outslice, col0, ncol):
            o0, o1 = outslice
            for c in range(3):
                nc.gpsimd.dma_start(t[c][:, col0:col0 + ncol],
                                    w_d[cv][c, 0:128, o0:o1])
            nc.gpsimd.dma_start(t[3][0:64, col0:col0 + ncol],
                                w_d[cv][0, 128:192, o0:o1])
            nc.gpsimd.dma_start(t[3][64:128, col0:col0 + ncol],
                                w_d[cv][1, 128:192, o0:o1])
            nc.gpsimd.dma_start(t[4][64:128, col0:col0 + ncol],
                                w_d[cv][2, 128:192, o0:o1])

        wt = {}
        for name in ("T0", "T1", "T2", "C0", "C1"):
            wt[name] = [const.tile([128, 128], F32R, tag=f"w{name}{c}",
                                   name=f"w{name}{c}") for c in range(5)]
            nc.vector.tensor_copy(wt[name][4][0:64, :], zsrc[:])
            if name == "C1":
                for c in range(5):
                    nc.vector.tensor_copy(wt[name][c][0:64, 64:128],
                                          zsrc[0:64, 0:64])
                    nc.vector.tensor_copy(wt[name][c][64:128, 64:128],
                                          zsrc[0:64, 0:64])
        load_chunk_rows(wt["T0"], "r", (0, 128), 0, 128)
        load_chunk_rows(wt["T1"], "g", (0, 128), 0, 128)
        load_chunk_rows(wt["T2"], "r", (128, 192), 0, 64)
        load_chunk_rows(wt["T2"], "g", (128, 192), 64, 64)
        load_chunk_rows(wt["C0"], "n", (0, 128), 0, 128)
        load_chunk_rows(wt["C1"], "n", (128, 192), 0, 64)
        # 64-row cand-chB tap-1/2 weight tiles at base partition 0 (matmul
        # requires lhsT and rhs to share a base partition)
        wc = {}
        for wname, (o0, o1), ncol in (("C0", (0, 128), 128),
                                      ("C1", (128, 192), 64)):
            for k in (1, 2):
                t = const.tile([64, 128], F32R, tag=f"wc{wname}{k}",
                               name=f"wc{wname}{k}")
                if ncol < 128:
                    nc.vector.tensor_copy(t[:, 64:128], zsrc[0:64, 0:64])
                nc.gpsimd.dma_start(t[:, 0:ncol], w_d["n"][k, 128:192, o0:o1])
                wc[wname, k] = t

        # --- bias tiles --------------------------------------------------
        bA = const.tile([128, 1], F32, tag="bA")
        nc.sync.dma_start(bA[:, 0], b_d["r"][0:128])
        bG = const.tile([128, 1], F32, tag="bG")
        nc.sync.dma_start(bG[:, 0], b_d["g"][0:128])
        bB = const.tile([128, 1], F32, tag="bB")
        nc.sync.dma_start(bB[0:64, 0], b_d["r"][128:192])
        nc.sync.dma_start(bB[64:128, 0], b_d["g"][128:192])
        bCA = const.tile([128, 1], F32, tag="bCA")
        nc.sync.dma_start(bCA[:, 0], b_d["n"][0:128])
        bCB = const.tile([128, 1], F32, tag="bCB")
        nc.sync.dma_start(bCB[0:64, 0], b_d["n"][128:192])

        ident = const.tile([128, 128], F32, tag="ident")
        nc.sync.dma_start(ident[:], id_d)
        identr = const.tile([128, 128], F32R, tag="identr")
        nc.gpsimd.dma_start(identr[:], id_d)

        # --- per-pipeline state tiles ------------------------------------
        memA, memB = {}, {}
        rmemA, rmemB = {}, {}
        zf32 = state.tile([128, WPADP], F32, tag="zf32", name="zf32")
        nc.gpsimd.memset(zf32[:], 0.0)
        for p in range(NP):
            for i in range(2):
                memA[p, i] = state.tile([128, WPADP], F32R, tag=f"memA{p}{i}",
                                        name=f"memA{p}{i}")
                memB[p, i] = state.tile([128, WPADP], F32R, tag=f"memB{p}{i}",
                                        name=f"memB{p}{i}")
            rmemA[p] = state.tile([128, WPADP], F32R, tag=f"rmemA{p}",
                                  name=f"rmemA{p}")
            rmemB[p] = state.tile([128, WPADP], F32R, tag=f"rmemB{p}",
                                  name=f"rmemB{p}")
            for t in (memA[p, 0], memA[p, 1], memB[p, 0], memB[p, 1],
                      rmemA[p], rmemB[p]):
                nc.vector.tensor_copy(t[:], zf32[:])

        # --- input transform: x[b,l,c] -> mem[p][c, 2 + 2l + j] ----------
        for b in range(BLOC):
            p, j = divmod(b, 2)
            xb = tmp.tile([L, C], F32, tag="xload")
            nc.sync.dma_start(xb[:], x_d[b])
            ps = psum.tile([128, L], F32, tag="tpF32")
            nc.tensor.transpose(ps[:], xb[:, 0:128], ident[:])
            nc.vector.tensor_copy(memA[p, 0][:, 2 + j: 2 + j + 2 * L: 2], ps[:])
            ps2 = psum.tile([128, L], F32, tag="tpF32")
            nc.tensor.transpose(ps2[0:64, :], xb[:, 128:192], ident[:])
            nc.vector.tensor_copy(memB[p, 0][0:64, 2 + j: 2 + j + 2 * L: 2],
                                  ps2[0:64, :])
        for p in range(NP):
            nc.vector.tensor_copy(memB[p, 0][64:128, 0:TOKP],
                                  memB[p, 0][0:64, 2:2 + TOKP])
        # uB cols 0:2 are the zero shift-in; zero them once in both rotating
        # buffers (the per-step mul only writes cols 2:TOKP)
        for p in range(NP):
            for _ in range(2):
                uBz = tmp.tile([128, TOKP], F32R, tag=f"uB{p}", name=f"uBz{p}")
                nc.vector.tensor_copy(uBz[64:128, 0:2], zf32[64:128, 0:2])

        # --- recurrence --------------------------------------------------
        MWIN = ((0, 0), (1, 2), (2, 4))

        cur = 0
        pt = {}   # psum tiles per pipeline, this step
        sig = {}  # activation tiles per pipeline

        def emit_rg(p):
            mA, mB = memA[p, cur], memB[p, cur]
            pT0 = psum.tile([128, TOKP], F32, tag=f"pT0_{p}", name=f"pT0_{p}")
            pT1 = psum.tile([128, TOKP], F32, tag=f"pT1_{p}", name=f"pT1_{p}")
            pT2 = psum.tile([128, TOKP], F32, tag=f"pT2_{p}", name=f"pT2_{p}")
            pt[p] = (pT0, pT1, pT2)

            def cha(pp, wts):
                for c, off in MWIN:
                    nc.tensor.matmul(pp[:], wts[c][:], mA[:, off:off + TOKP],
                                     start=(c == 0), stop=False)

            def chb(pp, wts):
                nc.tensor.matmul(pp[:], wts[3][:], mB[:, 0:TOKP],
                                 start=False, stop=False)
                nc.tensor.matmul(pp[:], wts[4][:], mB[:, 2:2 + TOKP],
                                 start=False, stop=True)

            cha(pT0, wt["T0"])   # reset-lo completes at mm 8 -> sA
            cha(pT2, wt["T2"])   # hi (reset|gate) at mm 10 -> sBG
            chb(pT0, wt["T0"])
            chb(pT2, wt["T2"])
            cha(pT1, wt["T1"])   # gate-lo last -> sG
            chb(pT1, wt["T1"])

        def emit_sig(p):
            pT0, pT1, pT2 = pt[p]
            sA = act.tile([128, TOKP], F32R, tag=f"sA{p}", name=f"sA{p}")
            nc.scalar.activation(sA[:], pT0[:], AF.Sigmoid, bias=bA[:, 0:1])
            # merged hi sigmoid: rows 0:64 = reset-hi, rows 64:128 = gate-hi
            sBG = act.tile([128, TOKP], F32R, tag=f"sBG{p}", name=f"sBG{p}")
            nc.scalar.activation(sBG[:], pT2[:], AF.Sigmoid, bias=bB[:, 0:1])
            sG = act.tile([128, TOKP], F32R, tag=f"sG{p}", name=f"sG{p}")
            nc.scalar.activation(sG[:], pT1[:], AF.Sigmoid, bias=bG[:, 0:1])
            sig[p] = (sA, sBG, sG)

        def emit_rmem(p):
            sA, sBG, _ = sig[p]
            mA, mB = memA[p, cur], memB[p, cur]
            nc.gpsimd.tensor_mul(rmemA[p][:, 2:2 + TOKP], sA[:],
                                 mA[:, 2:2 + TOKP])
            # main rows only: rmemB[0:64, 2+c] = sBr[c] * memB[0:64, 2+c]
            nc.vector.tensor_mul(rmemB[p][0:64, 2:2 + TOKP], sBG[0:64, :],
                                 mB[0:64, 2:2 + TOKP])

        def emit_u(p):
            _, sBG, sG = sig[p]
            mA, mB = memA[p, cur], memB[p, cur]
            uA = tmp.tile([128, TOKP], F32R, tag=f"uA{p}", name=f"uA{p}")
            nc.gpsimd.tensor_mul(uA[:], sG[:], mA[:, 0:TOKP])
            # uB on partitions 64:128: uB[c] = sGb[c] * mem_main[c]; main[c]
            # = dup[c-2], cols 0:2 are the zero shift-in (memset at init)
            uB = tmp.tile([128, TOKP], F32R, tag=f"uB{p}", name=f"uB{p}")
            nc.gpsimd.tensor_mul(uB[64:128, 2:TOKP], sBG[64:128, 2:TOKP],
                                 mB[64:128, 0:TOKP - 2])
            return uA, uB

        def emit_cand(p, hi_first=False):
            pC0 = psum.tile([128, TOKP], F32, tag=f"pT0_{p}", name=f"pC0_{p}")
            pC1 = psum.tile([128, TOKP], F32, tag=f"pT1_{p}", name=f"pC1_{p}")

            def full(pp, wts, wname):
                for c, off in MWIN:
                    nc.tensor.matmul(pp[:], wts[c][:],
                                     rmemA[p][:, off:off + TOKP],
                                     start=(c == 0), stop=False)
                # chB taps as three 64-contraction matmuls on the main rows
                nc.tensor.matmul(pp[:], wts[3][0:64, :],
                                 rmemB[p][0:64, 0:TOKP],
                                 start=False, stop=False)
                nc.tensor.matmul(pp[:], wc[wname, 1][:],
                                 rmemB[p][0:64, 2:2 + TOKP],
                                 start=False, stop=False)
                nc.tensor.matmul(pp[:], wc[wname, 2][:],
                                 rmemB[p][0:64, 4:4 + TOKP],
                                 start=False, stop=True)

            if hi_first:
                full(pC1, wt["C1"], "C1")
                full(pC0, wt["C0"], "C0")
            else:
                full(pC0, wt["C0"], "C0")
                full(pC1, wt["C1"], "C1")
            return pC0, pC1

        def emit_tanh_a(p, pC0):
            cA = act.tile([128, TOKP], F32R, tag=f"cA{p}", name=f"cA{p}")
            nc.scalar.activation(cA[:], pC0[:], AF.Tanh, bias=bCA[:, 0:1])
            return cA

        def emit_tanh_b(p, pC1):
            # written at base 64 so the B-side combine stays on 64:128
            cB = act.tile([128, TOKP], F32R, tag=f"cB{p}", name=f"cB{p}")
            nc.scalar.activation(cB[64:128, :], pC1[0:64, :], AF.Tanh,
                                 bias=bCB[0:64, 0:1])
            return cB

        def emit_combine_a(p, uA, cA):
            sG = sig[p][2]
            nA = memA[p, 1 - cur]
            qA = tmp.tile([128, TOKP], F32R, tag=f"qA{p}", name=f"qA{p}")
            nc.vector.scalar_tensor_tensor(qA[:], sG[:], 1.0, cA[:],
                                           op0=SUB, op1=MULT)
            nc.vector.tensor_sub(nA[:, 2:2 + TOKP], uA[:], qA[:])

        def emit_combine_b(p, uB, cB):
            sBG = sig[p][1]
            nB = memB[p, 1 - cur]
            qB = tmp.tile([128, TOKP], F32R, tag=f"qB{p}", name=f"qB{p}")
            nc.vector.scalar_tensor_tensor(qB[64:128, :], sBG[64:128, :], 1.0,
                                           cB[64:128, :], op0=SUB, op1=MULT)
            # write the DUP rows directly (same column indexing as main):
            nc.vector.tensor_sub(nB[64:128, 0:TOKP], uB[64:128, :],
                                 qB[64:128, :])
            # main rows from dup rows (unshift by 2 cols) on the idle DMA
            # engines, off the DVE queue: nB[0:64, c+2] = nB[64:128, c].
            nc.sync.dma_start(nB[0:64, 2:2 + TOKP], nB[64:128, 0:TOKP])

        for t in range(steps):
            emit_rg(0)
            emit_rg(1)
            emit_sig(0)
            emit_rmem(0)
            emit_sig(1)
            emit_rmem(1)
            u0 = emit_u(0)
            pC0_0, pC1_0 = emit_cand(0)
            cA0 = emit_tanh_a(0, pC0_0)
            cB0 = emit_tanh_b(0, pC1_0)
            emit_combine_a(0, u0[0], cA0)
            emit_combine_b(0, u0[1], cB0)
            u1 = emit_u(1)
            pC0_1, pC1_1 = emit_cand(1, hi_first=True)
            cB1 = emit_tanh_b(1, pC1_1)
            cA1 = emit_tanh_a(1, pC0_1)
            emit_combine_b(1, u1[1], cB1)
            emit_combine_a(1, u1[0], cA1)
            cur = 1 - cur

        # --- output transform -------------------------------------------
        for b in range(BLOC):
            p, j = divmod(b, 2)
            osb = tmp.tile([L, C], F32, tag="oload")
            ps = psum.tile([L, 128], F32R, tag="tpR")
            nc.tensor.transpose(ps[:], memA[p, cur][:, 2 + j: 2 + j + 2 * L: 2],
                                identr[:])
            nc.vector.tensor_copy(osb[:, 0:128], ps[:])
            ps2 = psum.tile([L, 128], F32R, tag="tpR")
            nc.tensor.transpose(ps2[:, 0:64],
                                memB[p, cur][0:64, 2 + j: 2 + j + 2 * L: 2],
                                identr[0:64, 0:64])
            nc.vector.tensor_copy(osb[:, 128:192], ps2[:, 0:64])
            nc.sync.dma_start(out_d[b], osb[:])

    nc.compile()
    return nc


_built = {}


def _get(steps=STEPS):
    if steps not in _built:
        _built[steps] = build(steps)
    return _built[steps]


def kernel(x, w_reset, b_reset, w_gate, b_gate, w_cand, b_cand, steps=STEPS,
           trace=False):
    nc = _get(steps)
    ident = np.eye(128, dtype=np.float32)
    base = {"w_reset": np.asarray(w_reset, np.float32),
            "b_reset": np.asarray(b_reset, np.float32),
            "w_gate": np.asarray(w_gate, np.float32),
            "b_gate": np.asarray(b_gate, np.float32),
            "w_cand": np.asarray(w_cand, np.float32),
            "b_cand": np.asarray(b_cand, np.float32),
            "ident": ident}
    x = np.asarray(x, np.float32)
    in_maps = [dict(base, x=np.ascontiguousarray(x[i * BLOC:(i + 1) * BLOC]))
               for i in range(NCORES)]
    res = run_bass_kernel_spmd(nc, in_maps, core_ids=list(range(NCORES)),
                               trace=trace)
    out = np.concatenate([res.results[i]["out"] for i in range(NCORES)], axis=0)
    if trace:
        return out, res
    return out
